# revision 5
# baseline (speedup 1.0000x reference)
"""Trainium2 Bass kernel: causal GQA prompt attention + paged KV-cache write.

Sharding: one KV head per NeuronCore (8 cores x 1 kv-head x 4 q-heads).
Flash-style attention in "transposed score" layout per core:
  scores_T[k, q] = K_T_i.T @ Q_T          (bf16 matmul, psum fp32)
  P_T = exp(SCALE * scores_T)             (ACT, bf16 out, causal-sliced)
  out_T[d, q]  += V_i.T @ P_T_i           (bf16 matmul accumulate)
  sums[q]      += ones.T @ P_T_i          (replicated row-sums)
  out = transpose(out_T * 1/sums)         (PE transpose, DVE normalize)
KV-cache write is a DRAM->DRAM DMA (slot_mapping handled host-side).
"""
import sys

sys.path.insert(0, "/opt/trn_rl_repo")

import numpy as np

import concourse.bass as bass
import concourse.mybir as mybir
import concourse.tile as tile
from concourse import bacc, bass_utils
from concourse.masks import make_identity, make_upper_triangular

B, S, H, HKV, D = 4, 1024, 32, 8, 128
G = H // HKV                      # q heads per kv head (= per core)
NCORES = 8
NT = S // 128                     # 8 k/q tiles of 128
SCALE = 0.08838834764831845
BLOCK_SIZE, NUM_BLOCKS = 128, 64
F32, BF16 = mybir.dt.float32, mybir.dt.bfloat16
Exp = mybir.ActivationFunctionType.Exp

# --- causal packing tables ----------------------------------------------------
# per chunk c (q in [512c, 512c+512)): psum groups -> (ktile i, col off, width);
# every slice sits inside one 512-col psum bank; groups sized <= 1024 (2 banks).
PACK = {
    0: [[(0, 0, 512), (1, 512, 384), (3, 896, 128)], [(2, 0, 256)]],
    1: [
        [(0, 0, 512), (1, 512, 512)],
        [(2, 0, 512), (3, 512, 512)],
        [(4, 0, 512), (5, 512, 384), (7, 896, 128)],
        [(6, 0, 256)],
    ],
}
EXTENT = {0: [1024, 256], 1: [1024, 1024, 1024, 256]}


def qstart(i, c):
    return max(128 * i, 512 * c)


def build_bass():
    nc = bacc.Bacc("TRN2", target_bir_lowering=False)
    q_in = nc.dram_tensor("q_s", [B, S, G * D], F32, kind="ExternalInput")
    k_in = nc.dram_tensor("k_s", [B, S, D], F32, kind="ExternalInput")
    v_in = nc.dram_tensor("v_s", [B, S, D], F32, kind="ExternalInput")
    o_out = nc.dram_tensor("o_s", [B, S, G * D], F32, kind="ExternalOutput")
    kc_out = nc.dram_tensor("kc_s", [NUM_BLOCKS, BLOCK_SIZE, D], F32, kind="ExternalOutput")
    vc_out = nc.dram_tensor("vc_s", [NUM_BLOCKS, BLOCK_SIZE, D], F32, kind="ExternalOutput")

    with tile.TileContext(nc) as tc:
        with (
            tc.tile_pool(name="const", bufs=1) as constp,
            tc.tile_pool(name="kv", bufs=2) as kvp,
            tc.tile_pool(name="qq", bufs=2) as qp,
            tc.tile_pool(name="stage", bufs=6) as stp,
            tc.tile_pool(name="outn", bufs=4) as onp,
            tc.tile_pool(name="ps_sc", bufs=2, space="PSUM") as psb,
            tc.tile_pool(name="ps_out", bufs=2, space="PSUM") as pso,
            tc.tile_pool(name="ps_sm", bufs=2, space="PSUM") as psa,
        ):
            ident = constp.tile([128, 128], F32, tag="ident")
            make_identity(nc, ident[:])
            tri = constp.tile([128, 128], BF16, tag="tri")
            make_upper_triangular(nc, tri[:], val=1.0, diag=True)
            ones = constp.tile([128, 128], BF16, tag="ones")
            nc.vector.memset(ones[:], 1.0)

            # paged KV-cache write (identity slot_mapping -> straight copy)
            nc.sync.dma_start(
                kc_out[:].rearrange("nb bs d -> (nb bs) d")[0 : B * S, :],
                k_in[:].rearrange("b s d -> (b s) d"),
            )
            nc.sync.dma_start(
                vc_out[:].rearrange("nb bs d -> (nb bs) d")[0 : B * S, :],
                v_in[:].rearrange("b s d -> (b s) d"),
            )

            for b in range(B):
                # --- K/V prep for this batch (shared by 4 q-heads) ---
                k_bf = kvp.tile([128, NT, 128], BF16, tag="k_bf")
                nc.gpsimd.dma_start(k_bf[:], k_in[b].rearrange("(t p) d -> p t d", p=128))
                v_bf = kvp.tile([128, NT, 128], BF16, tag="v_bf")
                nc.gpsimd.dma_start(v_bf[:], v_in[b].rearrange("(t p) d -> p t d", p=128))
                k_T = kvp.tile([128, NT, 128], BF16, tag="k_T")  # [d, t, k]
                nc.sync.dma_start(k_T[:], k_bf[:].rearrange("p t d -> p (t d)"), transpose=True)

                for h in range(G):
                    q_bf = qp.tile([128, NT, 128], BF16, tag="q_bf")
                    nc.gpsimd.dma_start(
                        q_bf[:],
                        q_in[b, :, h * D : (h + 1) * D].rearrange("(t p) d -> p t d", p=128),
                    )
                    q_T = qp.tile([128, NT, 128], BF16, tag="q_T")  # [d, t, q]
                    nc.sync.dma_start(q_T[:], q_bf[:].rearrange("p t d -> p (t d)"), transpose=True)
                    q_Tf = q_T[:].rearrange("d t q -> d (t q)")

                    for c in range(2):
                        stages = []  # (stage_tile, group)
                        for g_idx, group in enumerate(PACK[c]):
                            ext = EXTENT[c][g_idx]
                            sc_ps = psb.tile([128, 1024], F32, tag="scores")
                            for (i, off, w) in group:
                                qs = qstart(i, c)
                                nc.tensor.matmul(
                                    sc_ps[:, off : off + w],
                                    k_T[:, i, :],
                                    q_Tf[:, qs : qs + w],
                                    start=True, stop=True,
                                )
                            stg = stp.tile([128, 1024], BF16, tag="stage")
                            nc.scalar.activation(stg[:, 0:ext], sc_ps[:, 0:ext], Exp, scale=SCALE)
                            for (i, off, w) in group:
                                if i >= 4 * c:  # diagonal tile: mask strictly-lower (q<k)
                                    eng = nc.vector if i % 2 == 0 else nc.gpsimd
                                    eng.tensor_mul(
                                        stg[:, off : off + 128], stg[:, off : off + 128], tri[:]
                                    )
                            stages.append((stg, group))

                        out_ps = pso.tile([128, 512], F32, tag="out")
                        sums_ps = psa.tile([128, 512], F32, tag="sm")
                        ktiles = [(i, off, w, stg) for stg, grp in stages for (i, off, w) in grp]
                        ktiles.sort(key=lambda x: x[0])
                        last = ktiles[-1][0]
                        for (i, off, w, stg) in ktiles:
                            qr = qstart(i, c) - 512 * c
                            nc.tensor.matmul(
                                sums_ps[:, qr : qr + w], ones[:], stg[:, off : off + w],
                                start=(i == 0), stop=(i == last), skip_group_check=True,
                            )
                        for (i, off, w, stg) in ktiles:
                            qr = qstart(i, c) - 512 * c
                            nc.tensor.matmul(
                                out_ps[:, qr : qr + w], v_bf[:, i, :], stg[:, off : off + w],
                                start=(i == 0), stop=(i == last), skip_group_check=True,
                            )

                        recip = onp.tile([128, 512], F32, tag="recip")
                        nc.vector.reciprocal_approx_fast(recip[:], sums_ps[:])
                        out_n = onp.tile([128, 512], F32, tag="out_n")
                        nc.vector.tensor_mul(out_n[:], out_ps[:], recip[:])

                        ot_ps = psa.tile([128, 512], F32, tag="sm")
                        for t in range(4):
                            nc.tensor.transpose(
                                ot_ps[:, t * 128 : (t + 1) * 128],
                                out_n[:, t * 128 : (t + 1) * 128],
                                ident[:],
                            )
                        obuf = onp.tile([128, 512], F32, tag="obuf")
                        nc.vector.tensor_copy(obuf[:], ot_ps[:])
                        nc.sync.dma_start(
                            o_out[b, 512 * c : 512 * (c + 1), h * D : (h + 1) * D]
                            .rearrange("(t q) d -> q t d", q=128),
                            obuf[:].rearrange("q (t d) -> q t d", d=128),
                        )
    nc.compile()
    return nc


_NC = None


def kernel(query, key, value, key_cache, value_cache, slot_mapping):
    global _NC
    query = np.ascontiguousarray(np.asarray(query, np.float32))
    key = np.ascontiguousarray(np.asarray(key, np.float32))
    value = np.ascontiguousarray(np.asarray(value, np.float32))
    slot_np = np.asarray(slot_mapping)

    if _NC is None:
        _NC = build_bass()
    nc = _NC

    in_maps = []
    for h in range(NCORES):
        in_maps.append({
            "q_s": np.ascontiguousarray(query[:, :, h * G * D : (h + 1) * G * D]),
            "k_s": np.ascontiguousarray(key[:, :, h * D : (h + 1) * D]),
            "v_s": np.ascontiguousarray(value[:, :, h * D : (h + 1) * D]),
        })
    res = bass_utils.run_bass_kernel_spmd(nc, in_maps, core_ids=list(range(NCORES)))
    rs = res.results

    output = np.concatenate([rs[h]["o_s"] for h in range(NCORES)], axis=2)

    identity_slots = bool(np.array_equal(slot_np, np.arange(B * S)))
    kc = np.array(key_cache, np.float32).reshape(NUM_BLOCKS, BLOCK_SIZE, HKV, D).copy()
    vc = np.array(value_cache, np.float32).reshape(NUM_BLOCKS, BLOCK_SIZE, HKV, D).copy()
    bi, bo = slot_np // BLOCK_SIZE, slot_np % BLOCK_SIZE
    if identity_slots:
        kc_dev = np.stack([rs[h]["kc_s"] for h in range(NCORES)], axis=2)
        vc_dev = np.stack([rs[h]["vc_s"] for h in range(NCORES)], axis=2)
        kc[bi, bo] = kc_dev[bi, bo]
        vc[bi, bo] = vc_dev[bi, bo]
    else:  # general slot_mapping: host-side scatter (cache only)
        kc[bi, bo] = key.reshape(-1, HKV, D)
        vc[bi, bo] = value.reshape(-1, HKV, D)
    return output, kc, vc


# revision 6
# speedup vs baseline: 1.1027x; 1.1027x over previous
"""Trainium2 Bass kernel: causal GQA prompt attention + paged KV-cache write.

Sharding: one KV head per NeuronCore (8 cores x 1 kv-head x 4 q-heads).
Flash-style attention in "transposed score" layout per core:
  scores_T[k, q] = K_T_i.T @ Q_T          (bf16 matmul, psum fp32)
  P_T = exp(SCALE * scores_T)             (ACT, bf16 out, causal-sliced)
  out_T[d, q]  += V_i.T @ P_T_i           (bf16 matmul accumulate)
  sums[q]      += ones.T @ P_T_i          (replicated row-sums)
  out = transpose(out_T * 1/sums)         (PE transpose, DVE normalize)
KV-cache write is a DRAM->DRAM DMA (slot_mapping handled host-side).
"""
import sys

sys.path.insert(0, "/opt/trn_rl_repo")

import numpy as np

import concourse.bass as bass
import concourse.mybir as mybir
import concourse.tile as tile
from concourse import bacc, bass_utils
from concourse.masks import make_identity, make_upper_triangular

B, S, H, HKV, D = 4, 1024, 32, 8, 128
G = H // HKV                      # q heads per kv head (= per core)
NCORES = 8
NT = S // 128                     # 8 k/q tiles of 128
SCALE = 0.08838834764831845
BLOCK_SIZE, NUM_BLOCKS = 128, 64
F32, BF16 = mybir.dt.float32, mybir.dt.bfloat16
Exp = mybir.ActivationFunctionType.Exp

# --- causal packing tables ----------------------------------------------------
# per chunk c (q in [512c, 512c+512)): psum groups -> (ktile i, col off, width);
# every slice sits inside one 512-col psum bank; groups sized <= 1024 (2 banks).
PACK = {
    0: [[(0, 0, 512), (1, 512, 384), (3, 896, 128)], [(2, 0, 256)]],
    1: [
        [(0, 0, 512), (1, 512, 512)],
        [(2, 0, 512), (3, 512, 512)],
        [(4, 0, 512), (5, 512, 384), (7, 896, 128)],
        [(6, 0, 256)],
    ],
}
EXTENT = {0: [1024, 256], 1: [1024, 1024, 1024, 256]}


def qstart(i, c):
    return max(128 * i, 512 * c)


def build_bass():
    nc = bacc.Bacc("TRN2", target_bir_lowering=False)
    q_in = nc.dram_tensor("q_s", [B, S, G * D], F32, kind="ExternalInput")
    k_in = nc.dram_tensor("k_s", [B, S, D], F32, kind="ExternalInput")
    v_in = nc.dram_tensor("v_s", [B, S, D], F32, kind="ExternalInput")
    o_out = nc.dram_tensor("o_s", [B, S, G * D], F32, kind="ExternalOutput")
    kc_out = nc.dram_tensor("kc_s", [NUM_BLOCKS, BLOCK_SIZE, D], F32, kind="ExternalOutput")
    vc_out = nc.dram_tensor("vc_s", [NUM_BLOCKS, BLOCK_SIZE, D], F32, kind="ExternalOutput")

    with tile.TileContext(nc) as tc:
        with (
            tc.tile_pool(name="const", bufs=1) as constp,
            tc.tile_pool(name="kv", bufs=2) as kvp,
            tc.tile_pool(name="qq", bufs=2) as qp,
            tc.tile_pool(name="stage", bufs=6) as stp,
            tc.tile_pool(name="outn", bufs=4) as onp,
            tc.tile_pool(name="ps_sc", bufs=2, space="PSUM") as psb,
            tc.tile_pool(name="ps_out", bufs=2, space="PSUM") as pso,
            tc.tile_pool(name="ps_sm", bufs=2, space="PSUM") as psa,
        ):
            ident = constp.tile([128, 128], F32, tag="ident")
            make_identity(nc, ident[:])
            tri = constp.tile([128, 128], BF16, tag="tri")
            make_upper_triangular(nc, tri[:], val=1.0, diag=True)
            ones = constp.tile([128, 128], BF16, tag="ones")
            nc.vector.memset(ones[:], 1.0)

            # paged KV-cache write (identity slot_mapping -> straight copy)
            nc.sync.dma_start(
                kc_out[:].rearrange("nb bs d -> (nb bs) d")[0 : B * S, :],
                k_in[:].rearrange("b s d -> (b s) d"),
            )
            nc.sync.dma_start(
                vc_out[:].rearrange("nb bs d -> (nb bs) d")[0 : B * S, :],
                v_in[:].rearrange("b s d -> (b s) d"),
            )

            for b in range(B):
                # --- K/V prep for this batch (shared by 4 q-heads) ---
                k_bf = kvp.tile([128, NT, 128], BF16, tag="k_bf")
                nc.gpsimd.dma_start(k_bf[:], k_in[b].rearrange("(t p) d -> p t d", p=128))
                v_bf = kvp.tile([128, NT, 128], BF16, tag="v_bf")
                nc.gpsimd.dma_start(v_bf[:], v_in[b].rearrange("(t p) d -> p t d", p=128))
                k_T = kvp.tile([128, NT, 128], BF16, tag="k_T")  # [d, t, k]
                nc.sync.dma_start(k_T[:], k_bf[:].rearrange("p t d -> p (t d)"), transpose=True)

                for h in range(G):
                    q_bf = qp.tile([128, NT, 128], BF16, tag="q_bf")
                    nc.gpsimd.dma_start(
                        q_bf[:],
                        q_in[b, :, h * D : (h + 1) * D].rearrange("(t p) d -> p t d", p=128),
                    )
                    q_T = qp.tile([128, NT, 128], BF16, tag="q_T")  # [d, t, q]
                    nc.sync.dma_start(q_T[:], q_bf[:].rearrange("p t d -> p (t d)"), transpose=True)
                    q_Tf = q_T[:].rearrange("d t q -> d (t q)")

                    for c in range(2):
                        stages = []  # (stage_tile, group)
                        for g_idx, group in enumerate(PACK[c]):
                            ext = EXTENT[c][g_idx]
                            sc_ps = psb.tile([128, 1024], F32, tag="scores")
                            for (i, off, w) in group:
                                qs = qstart(i, c)
                                nc.tensor.matmul(
                                    sc_ps[:, off : off + w],
                                    k_T[:, i, :],
                                    q_Tf[:, qs : qs + w],
                                    start=True, stop=True,
                                )
                            stg = stp.tile([128, 1024], BF16, tag="stage")
                            nc.scalar.activation(stg[:, 0:ext], sc_ps[:, 0:ext], Exp, scale=SCALE)
                            for (i, off, w) in group:
                                if i >= 4 * c:  # diagonal tile: mask strictly-lower (q<k)
                                    nc.vector.tensor_mul(
                                        stg[:, off : off + 128], stg[:, off : off + 128], tri[:]
                                    )
                            stages.append((stg, group))

                        out_ps = pso.tile([128, 512], F32, tag="out")
                        sums_ps = psa.tile([128, 512], F32, tag="sm")
                        ktiles = [(i, off, w, stg) for stg, grp in stages for (i, off, w) in grp]
                        ktiles.sort(key=lambda x: x[0])
                        last = ktiles[-1][0]
                        for (i, off, w, stg) in ktiles:
                            qr = qstart(i, c) - 512 * c
                            nc.tensor.matmul(
                                sums_ps[:, qr : qr + w], ones[:], stg[:, off : off + w],
                                start=(i == 0), stop=(i == last), skip_group_check=True,
                            )
                        for (i, off, w, stg) in ktiles:
                            qr = qstart(i, c) - 512 * c
                            nc.tensor.matmul(
                                out_ps[:, qr : qr + w], v_bf[:, i, :], stg[:, off : off + w],
                                start=(i == 0), stop=(i == last), skip_group_check=True,
                            )

                        recip = onp.tile([128, 512], F32, tag="recip")
                        nc.vector.reciprocal_approx_fast(recip[:], sums_ps[:])
                        out_n = onp.tile([128, 512], F32, tag="out_n")
                        nc.vector.tensor_mul(out_n[:], out_ps[:], recip[:])

                        ot_ps = psa.tile([128, 512], F32, tag="sm")
                        for t in range(4):
                            nc.tensor.transpose(
                                ot_ps[:, t * 128 : (t + 1) * 128],
                                out_n[:, t * 128 : (t + 1) * 128],
                                ident[:],
                            )
                        obuf = onp.tile([128, 512], F32, tag="obuf")
                        nc.vector.tensor_copy(obuf[:], ot_ps[:])
                        nc.sync.dma_start(
                            o_out[b, 512 * c : 512 * (c + 1), h * D : (h + 1) * D]
                            .rearrange("(t q) d -> q t d", q=128),
                            obuf[:].rearrange("q (t d) -> q t d", d=128),
                        )
    nc.compile()
    return nc


_NC = None


def kernel(query, key, value, key_cache, value_cache, slot_mapping):
    global _NC
    query = np.ascontiguousarray(np.asarray(query, np.float32))
    key = np.ascontiguousarray(np.asarray(key, np.float32))
    value = np.ascontiguousarray(np.asarray(value, np.float32))
    slot_np = np.asarray(slot_mapping)

    if _NC is None:
        _NC = build_bass()
    nc = _NC

    in_maps = []
    for h in range(NCORES):
        in_maps.append({
            "q_s": np.ascontiguousarray(query[:, :, h * G * D : (h + 1) * G * D]),
            "k_s": np.ascontiguousarray(key[:, :, h * D : (h + 1) * D]),
            "v_s": np.ascontiguousarray(value[:, :, h * D : (h + 1) * D]),
        })
    res = bass_utils.run_bass_kernel_spmd(nc, in_maps, core_ids=list(range(NCORES)))
    rs = res.results

    output = np.concatenate([rs[h]["o_s"] for h in range(NCORES)], axis=2)

    identity_slots = bool(np.array_equal(slot_np, np.arange(B * S)))
    kc = np.array(key_cache, np.float32).reshape(NUM_BLOCKS, BLOCK_SIZE, HKV, D).copy()
    vc = np.array(value_cache, np.float32).reshape(NUM_BLOCKS, BLOCK_SIZE, HKV, D).copy()
    bi, bo = slot_np // BLOCK_SIZE, slot_np % BLOCK_SIZE
    if identity_slots:
        kc_dev = np.stack([rs[h]["kc_s"] for h in range(NCORES)], axis=2)
        vc_dev = np.stack([rs[h]["vc_s"] for h in range(NCORES)], axis=2)
        kc[bi, bo] = kc_dev[bi, bo]
        vc[bi, bo] = vc_dev[bi, bo]
    else:  # general slot_mapping: host-side scatter (cache only)
        kc[bi, bo] = key.reshape(-1, HKV, D)
        vc[bi, bo] = value.reshape(-1, HKV, D)
    return output, kc, vc
